# revision 9
# baseline (speedup 1.0000x reference)
"""Trainium2 Bass kernel for AttentionDenseBlock (SE gate + offset conv + deform conv + tanh).

Strategy (per core, data-parallel over batch: 1 sample/core on 8 cores):
  - SE gate: spatial mean -> fc1 -> relu -> fc2 -> sigmoid -> channel scale.
  - Offset conv: 9 shifted bf16 matmuls accumulating in PSUM.
  - Deform conv: bilinear sampling with |offset|<1 decomposes EXACTLY into a
    static 3x3-tap stencil per kernel position with data-dependent weights
    wy in {relu(-dy), 1-|dy|, relu(dy)} (x) wx likewise.  Each of the 81
    (k, r, s) terms is:  out += W_k @ (map_t (*) xs_shifted).
  - Perf structure:
    * tap maps stored k-major in DRAM per quarter; ONE batched
      partition-broadcast DMA per (k, quarter).
    * two bf16 copies of the padded input (even/odd column phase) keep every
      DVE modulation multiply 4B-aligned -> 2x_1P mode.
    * Z_KS kernel positions accumulate their 9 taps on DVE (z-tiles) and hit
      the PE once; their DVE chains are interleaved into the per-tap multiply
      stream so the PE never starves on them.
    * preamble (offset conv -> tap maps) for quarter q+1 is issued between
      main-loop quarters so it overlaps; PE stays warm.
  - Epilogue: tanh(psum + b_conv) fused on ACT -> DMA out.
"""

import os
import sys
from contextlib import ExitStack

import numpy as np

sys.path.insert(0, "/opt/trn_rl_repo")

import concourse.bass as bass
import concourse.bacc as bacc
import concourse.mybir as mybir
import concourse.tile as tile
from concourse.masks import make_identity

B, C, O, H, W = 8, 256, 256, 56, 56
KH = KW = 3
K2 = 9
HP, WP = H + 4, W + 4  # zero-padded by 2 for the 5x5 shift range
HW = H * W
QROWS = 14            # rows per quarter
QN = HW // 4          # 784 spatial positions per quarter
NN = QN // 2          # 392 = matmul N-chunk (fits one PSUM bank)
RED = 16              # SE reduction dim

# kernel positions handled via z-accumulation (DVE) instead of per-tap matmuls
Z_KS = (0, 8)
Z_DVE_TAPS = 9   # all z adds on DVE
TAP_KS = tuple(k for k in range(K2) if k not in Z_KS)

F32 = mybir.dt.float32
BF16 = mybir.dt.bfloat16
AF = mybir.ActivationFunctionType
ALU = mybir.AluOpType

LAST_RESULT = None


def _bcast_ap(base, extra_dims):
    """AP reading `base` ([128, N]) with extra broadcast/reshape free dims."""
    return bass.AP(tensor=base.tensor, offset=base.offset,
                   ap=[list(base.ap[0])] + [list(d) for d in extra_dims])


def build():
    nc = bacc.Bacc()
    x_d = nc.dram_tensor("x", (1, C, H, W), F32, kind="ExternalInput")
    woff_d = nc.dram_tensor("w_off", (2 * K2, C, KH, KW), F32, kind="ExternalInput")
    boff_d = nc.dram_tensor("b_off", (2 * K2,), F32, kind="ExternalInput")
    wconv_d = nc.dram_tensor("w_conv", (O, C, KH, KW), F32, kind="ExternalInput")
    bconv_d = nc.dram_tensor("b_conv", (O,), F32, kind="ExternalInput")
    fc1_d = nc.dram_tensor("fc1", (RED, C), F32, kind="ExternalInput")
    fc2_d = nc.dram_tensor("fc2", (C, RED), F32, kind="ExternalInput")
    out_d = nc.dram_tensor("out", (1, O, H, W), F32, kind="ExternalOutput")

    with tile.TileContext(nc) as tc, ExitStack() as ctx:
        singles = ctx.enter_context(tc.tile_pool(name="singles", bufs=1))
        wyxpool = ctx.enter_context(tc.tile_pool(name="wyxpool", bufs=2))
        mpool = ctx.enter_context(tc.tile_pool(name="mpool", bufs=5))
        mgpool = ctx.enter_context(tc.tile_pool(name="mgpool", bufs=6))
        reppool = ctx.enter_context(tc.tile_pool(name="reppool", bufs=3))
        zreppool = ctx.enter_context(tc.tile_pool(name="zreppool", bufs=1))
        zpool = ctx.enter_context(tc.tile_pool(name="zpool", bufs=2))
        outpool = ctx.enter_context(tc.tile_pool(name="outpool", bufs=2))
        mappool = ctx.enter_context(tc.tile_pool(name="mappool", bufs=2))
        dpool = ctx.enter_context(tc.tile_pool(name="dpool", bufs=1, space="DRAM"))
        psum_pre = ctx.enter_context(tc.tile_pool(name="psum_pre", bufs=1, space="PSUM"))
        psum_main = ctx.enter_context(tc.tile_pool(name="psum_main", bufs=1, space="PSUM"))

        # ---- static tiles ----
        xs_bf = singles.tile([128, 2, HP, WP], BF16)     # padded, scaled, even phase
        xs_bf1 = singles.tile([128, 2, HP, WP], BF16)    # odd phase: col c = col c+1
        wT = singles.tile([128, 2, K2, O], BF16)         # [c, cc, k, o]
        # woffT free dim: [0:9]=dy weights, [32:41]=dx weights (aligned blocks)
        woffT = singles.tile([128, 2, K2, 64], BF16)
        fc1T = singles.tile([128, 2, RED], F32)
        fc2T = singles.tile([128, C], F32)
        bconv = singles.tile([128, 2], F32)
        boff = singles.tile([64, 1], F32)
        y_se = singles.tile([128, 2, 1], F32)
        h_se = singles.tile([128, 1], F32)
        s_se = singles.tile([128, 2, 1], F32)
        boffn = singles.tile([64, 1], F32)
        ident = singles.tile([128, 128], BF16)
        wnat2 = singles.tile([128, 2, C * K2], BF16)
        woff_nat2 = singles.tile([2 * K2, C * K2], BF16)
        fc1Tb = singles.tile([128, 2, RED], F32)
        fc2Tb = singles.tile([128, C], F32)
        # k-major tap maps per quarter: row k*9 + (3r+s)
        maps_dram = [dpool.tile([K2 * K2, QN], BF16, name=f"maps{q}") for q in range(4)]

        make_identity(nc, ident[:, :])

        # ---- input DMA + weight DMA (cast to bf16 during DMA on SWDGE) ----
        nc.vector.memset(xs_bf[:, :, :, :], 0.0)
        x_r = x_d[:].rearrange("one c h w -> (one c) h w")
        for cc in range(2):
            nc.gpsimd.dma_start(out=xs_bf[:, cc, 2:2 + H, 2:2 + W],
                                in_=x_r[cc * 128:(cc + 1) * 128, :, :])
        wc_r = wconv_d[:].rearrange("o c kh kw -> o (c kh kw)")
        for oc in range(2):
            nc.gpsimd.dma_start(out=wnat2[:, oc, :], in_=wc_r[oc * 128:(oc + 1) * 128, :])
        nc.gpsimd.dma_start(out=woff_nat2[:, :],
                            in_=woff_d[:].rearrange("o c kh kw -> o (c kh kw)"))
        fc1_r = fc1_d[:].rearrange("m c -> c m")
        for cc in range(2):
            nc.sync.dma_start(out=fc1T[:, cc, :], in_=fc1_r[cc * 128:(cc + 1) * 128, :])
        nc.vector.memset(fc2T[:, :], 0.0)
        nc.sync.dma_start(out=fc2T[0:RED, :], in_=fc2_d[:].rearrange("c m -> m c"))
        nc.sync.dma_start(out=bconv[:, :],
                          in_=bconv_d[:].rearrange("(a c) -> c a", a=2))
        # b_off loaded de-interleaved: dy biases -> rows 0:9, dx -> rows 32:41
        nc.vector.memset(boff[:, :], 0.0)
        boff_src = boff_d[:]
        nc.sync.dma_start(out=boff[0:K2, 0:1],
                          in_=bass.AP(tensor=boff_src.tensor, offset=boff_src.offset,
                                      ap=[[2, K2], [0, 1]]))
        nc.sync.dma_start(out=boff[32:32 + K2, 0:1],
                          in_=bass.AP(tensor=boff_src.tensor,
                                      offset=boff_src.offset + 1,
                                      ap=[[2, K2], [0, 1]]))

        # ---- SE gate (mean over unscaled x, then scale xs in place) ----
        for cc in range(2):
            nc.vector.tensor_reduce(out=y_se[:, cc, 0:1], in_=xs_bf[:, cc, :, :],
                                    axis=mybir.AxisListType.XY, op=ALU.add)
        nc.vector.tensor_scalar_mul(y_se[:, :, 0:1], y_se[:, :, 0:1], 1.0 / HW)
        nc.vector.tensor_copy(fc1Tb[:, :, :], fc1T[:, :, :])
        nc.vector.tensor_copy(fc2Tb[:, :], fc2T[:, :])
        h_ps = psum_pre.tile([128, RED], F32, tag="se")
        for cc in range(2):
            nc.tensor.matmul(h_ps[0:RED, 0:1], lhsT=fc1Tb[:, cc, :], rhs=y_se[:, cc, 0:1],
                             start=(cc == 0), stop=(cc == 1))
        nc.vector.memset(h_se[:, :], 0.0)
        nc.vector.tensor_relu(h_se[0:RED, 0:1], h_ps[0:RED, 0:1])
        for cc in range(2):
            s_ps = psum_pre.tile([128, RED], F32, tag="se")
            nc.tensor.matmul(s_ps[:, 0:1], lhsT=fc2Tb[:, cc * 128:(cc + 1) * 128],
                             rhs=h_se[:, 0:1], start=True, stop=True)
            nc.scalar.activation(s_se[:, cc, 0:1], s_ps[:, 0:1], AF.Sigmoid)
        for cc in range(2):
            nc.vector.tensor_scalar_mul(xs_bf[:, cc, :, :], xs_bf[:, cc, :, :],
                                        s_se[:, cc, 0:1])
        # odd column phase (cols 0..57 used by tap windows; col 58/59 never read)
        nc.sync.dma_start(out=xs_bf1[:, :, :, 0:WP - 1], in_=xs_bf[:, :, :, 1:WP])

        nc.vector.memset(woffT[:, :, :, :], 0.0)

        # ---- transpose conv weights on PE: wT[c, cc, k, o] ----
        for kk in range(K2):
            for cc in range(2):
                for oc in range(2):
                    tp = psum_pre.tile([128, 128], BF16, tag="tp")
                    src = wnat2[:, oc, :].rearrange("p (c k) -> p c k", k=K2)
                    nc.tensor.transpose(tp[:, :], src[:, cc * 128:(cc + 1) * 128, kk],
                                        ident[:, :])
                    nc.vector.tensor_copy(wT[:, cc, kk, oc * 128:(oc + 1) * 128],
                                          tp[:, :])
                tp = psum_pre.tile([128, 128], BF16, tag="tp")
                srco = woff_nat2[:, :].rearrange("p (c k) -> p c k", k=K2)
                nc.tensor.transpose(tp[:, 0:2 * K2],
                                    srco[:, cc * 128:(cc + 1) * 128, kk],
                                    ident[0:2 * K2, 0:2 * K2])
                # de-interleave offset channels: dy -> cols 0:9, dx -> cols 32:41
                nc.vector.tensor_copy(woffT[:, cc, kk, 0:K2], tp[:, 0:2 * K2:2])
                nc.vector.tensor_copy(woffT[:, cc, kk, 32:32 + K2],
                                      tp[:, 1:2 * K2:2])

        nc.scalar.activation(boffn[:, 0:1], boff[:, 0:1], AF.Copy, scale=-1.0)

        # ---- per-quarter preamble: offset conv -> tap-weight maps -> DRAM ----
        def preamble(q):
            wy0 = wyxpool.tile([K2, QN], BF16, tag="wy0")
            wy1 = wyxpool.tile([K2, QN], BF16, tag="wy1")
            wy2 = wyxpool.tile([K2, QN], BF16, tag="wy2")
            wx0 = wyxpool.tile([K2, QN], BF16, tag="wx0")
            wx1 = wyxpool.tile([K2, QN], BF16, tag="wx1")
            wx2 = wyxpool.tile([K2, QN], BF16, tag="wx2")
            for nn in range(2):
                off_ps = psum_pre.tile([64, NN], F32, tag="off")
                for kk in range(K2):
                    ki, kj = divmod(kk, 3)
                    dh, dw = ki - 1, kj - 1
                    for cc in range(2):
                        r0 = 2 + dh + q * QROWS + nn * (QROWS // 2)
                        rhs = xs_bf[:, cc, r0:r0 + QROWS // 2, 2 + dw:2 + dw + W]
                        nc.tensor.matmul(off_ps[0:64, :],
                                         lhsT=woffT[:, cc, kk, 0:64], rhs=rhs,
                                         start=(kk == 0 and cc == 0),
                                         stop=(kk == K2 - 1 and cc == 1))
                # offset = psum + b_off, fused into relu(+-offset) tap weights
                nsl = slice(nn * NN, (nn + 1) * NN)
                nc.scalar.activation(wy0[:, nsl], off_ps[0:K2, :], AF.Relu,
                                     scale=-1.0, bias=boffn[0:K2, 0:1])
                nc.scalar.activation(wy2[:, nsl], off_ps[0:K2, :], AF.Relu,
                                     scale=1.0, bias=boff[0:K2, 0:1])
                nc.scalar.activation(wx0[:, nsl], off_ps[32:32 + K2, :], AF.Relu,
                                     scale=-1.0, bias=boffn[32:32 + K2, 0:1])
                nc.scalar.activation(wx2[:, nsl], off_ps[32:32 + K2, :], AF.Relu,
                                     scale=1.0, bias=boff[32:32 + K2, 0:1])
            # wy1 = 1 - |dy| = 1 - (relu(dy) + relu(-dy)); same for wx1
            nc.vector.tensor_add(wy1[:, :], wy0[:, :], wy2[:, :])
            nc.scalar.activation(wy1[:, :], wy1[:, :], AF.Copy, scale=-1.0, bias=1.0)
            nc.vector.tensor_add(wx1[:, :], wx0[:, :], wx2[:, :])
            nc.scalar.activation(wx1[:, :], wx1[:, :], AF.Copy, scale=-1.0, bias=1.0)
            wys = (wy0, wy1, wy2)
            wxs = (wx0, wx1, wx2)
            md = maps_dram[q][0:1, 0:1]
            for r in range(3):
                for s in range(3):
                    rs = 3 * r + s
                    mtmp = mappool.tile([K2, QN], BF16)
                    nc.vector.tensor_mul(mtmp[:, :], wys[r][:, :], wxs[s][:, :])
                    nc.sync.dma_start(
                        out=bass.AP(tensor=md.tensor, offset=md.offset + rs * QN,
                                    ap=[[K2 * QN, K2], [1, QN]]),
                        in_=mtmp[:, :])

        # ---- main deform-conv loop for one quarter ----
        n_events = len(TAP_KS) * K2 + len(Z_KS)

        def mod_mul(out_ap, mrep_row, kk, r, s, q, engine=None):
            ki, kj = divmod(kk, 3)
            dh, dw = ki - 1 + r - 1, kj - 1 + s - 1
            r0 = 2 + dh + q * QROWS
            cs = 2 + dw
            if cs % 2 == 0:
                xs_win = xs_bf[:, :, r0:r0 + QROWS, cs:cs + W]
            else:
                xs_win = xs_bf1[:, :, r0:r0 + QROWS, cs - 1:cs - 1 + W]
            mrep_b = _bcast_ap(mrep_row, [[0, 2], [W, QROWS], [1, W]])
            (engine or nc.vector).tensor_tensor(out_ap, xs_win, mrep_b, op=ALU.mult)

        def main_quarter(q):
            ps = [psum_main.tile([128, QN], F32, tag=f"ps{oc}", name=f"ps{oc}")
                  for oc in range(2)]

            def do_mms(rhs_tile, kk, ev):
                for cc in range(2):
                    for oc in range(2):
                        for n0, n1 in ((0, 512), (512, QN)):
                            nc.tensor.matmul(
                                ps[oc][:, n0:n1],
                                lhsT=wT[:, cc, kk, oc * 128:(oc + 1) * 128],
                                rhs=rhs_tile[:, cc, n0:n1],
                                start=(ev == 0 and cc == 0),
                                stop=(ev == n_events - 1 and cc == 1))

            md = maps_dram[q][0:1, 0:1]

            def bcast(out_ap, kk):
                nc.gpsimd.dma_start(
                    out=out_ap,
                    in_=bass.AP(tensor=md.tensor, offset=md.offset + kk * K2 * QN,
                                ap=[[0, 128], [QN, K2], [1, QN]]))

            # z-map broadcasts + per-tap k lookahead broadcast queue
            zrep = zreppool.tile([128, len(Z_KS), K2, QN], BF16, name="zrep")
            for zi, zk in enumerate(Z_KS):
                bcast(zrep[:, zi, :, :], zk)
            reps = {}
            reps[TAP_KS[0]] = reppool.tile([128, K2, QN], BF16, name="rep")
            bcast(reps[TAP_KS[0]][:, :, :], TAP_KS[0])

            # build deferred z-op list: [(is_first_mult?, zi, zk, r, s)]
            zaccs = {}
            for zi, zk in enumerate(Z_KS):
                zaccs[zi] = zpool.tile([128, 2, QN], BF16, name=f"zacc{zi}")
            zops = []
            for t in range(K2):
                for zi, zk in enumerate(Z_KS):
                    zops.append((zi, zk, t))

            def issue_zop(idx):
                zi, zk, t = zops[idx]
                r, s = divmod(t, 3)
                acc = zaccs[zi]
                if t == 0:
                    acc_v = acc[:, :, :].rearrange("p a (r c) -> p a r c", c=W)
                    mod_mul(acc_v, zrep[:, zi, t, :], zk, r, s, q)
                elif t < Z_DVE_TAPS:
                    m = mpool.tile([128, 2, QN], BF16)
                    m_v = m[:, :, :].rearrange("p a (r c) -> p a r c", c=W)
                    mod_mul(m_v, zrep[:, zi, t, :], zk, r, s, q)
                    nc.vector.tensor_add(acc[:, :, :], acc[:, :, :], m[:, :, :])
                else:
                    m = mgpool.tile([128, 2, QN], BF16, name="mg")
                    m_v = m[:, :, :].rearrange("p a (r c) -> p a r c", c=W)
                    mod_mul(m_v, zrep[:, zi, t, :], zk, r, s, q)
                    nc.vector.tensor_add(acc[:, :, :], acc[:, :, :], m[:, :, :])

            # interleave: hold z-ops for the first Z_DELAY taps (zrep DMA still
            # in flight), then catch up evenly so chains finish ~5 taps early
            Z_DELAY = 8
            n_taps = len(TAP_KS) * K2
            zdone = 0
            ev = 0
            tap_i = 0
            for ki, kk in enumerate(TAP_KS):
                if ki + 1 < len(TAP_KS):
                    nk = TAP_KS[ki + 1]
                    reps[nk] = reppool.tile([128, K2, QN], BF16, name="rep")
                    bcast(reps[nk][:, :, :], nk)
                for t in range(K2):
                    r, s = divmod(t, 3)
                    m = mpool.tile([128, 2, QN], BF16)
                    m_v = m[:, :, :].rearrange("p a (r c) -> p a r c", c=W)
                    mod_mul(m_v, reps[kk][:, t, :], kk, r, s, q)
                    do_mms(m, kk, ev)
                    ev += 1
                    tap_i += 1
                    if tap_i > Z_DELAY:
                        ztarget = min(len(zops),
                                      (len(zops) * (tap_i - Z_DELAY)) // (n_taps - Z_DELAY - 18) if n_taps - Z_DELAY - 18 > 0 else len(zops))
                        while zdone < ztarget:
                            issue_zop(zdone)
                            zdone += 1
            while zdone < len(zops):
                issue_zop(zdone)
                zdone += 1
            for zi, zk in enumerate(Z_KS):
                do_mms(zaccs[zi], zk, ev)
                ev += 1
            assert ev == n_events

            out_r = out_d[:].rearrange("one o h w -> (one o) h w")
            for oc in range(2):
                osb = outpool.tile([128, QN], F32)
                nc.scalar.activation(osb[:, :], ps[oc][:, :], AF.Tanh,
                                     bias=bconv[:, oc:oc + 1])
                nc.sync.dma_start(
                    out=out_r[oc * 128:(oc + 1) * 128,
                              q * QROWS:(q + 1) * QROWS, :],
                    in_=osb[:, :])

        # software pipeline: preamble(q+1) issued between main quarters
        preamble(0)
        preamble(1)
        main_quarter(0)
        preamble(2)
        main_quarter(1)
        preamble(3)
        main_quarter(2)
        main_quarter(3)
    nc.finalize()
    return nc


_NC = None


def _get_nc():
    global _NC
    if _NC is None:
        _NC = build()
    return _NC


def kernel(**inputs):
    global LAST_RESULT
    from concourse.bass_utils import run_bass_kernel_spmd

    nc = _get_nc()
    x = np.ascontiguousarray(inputs["x"], dtype=np.float32)
    shared = {k: np.ascontiguousarray(np.asarray(inputs[k]), dtype=np.float32)
              for k in ("w_off", "b_off", "w_conv", "b_conv", "fc1", "fc2")}
    in_maps = [{"x": x[i:i + 1], **shared} for i in range(B)]
    res = run_bass_kernel_spmd(nc, in_maps, core_ids=list(range(B)),
                               trace=bool(int(os.environ.get("KB_TRACE", "0"))))
    LAST_RESULT = res
    out = np.concatenate([res.results[i]["out"] for i in range(B)], axis=0)
    return out.astype(np.float32)


if __name__ == "__main__":
    nc = build()
    print("build OK")


# revision 10
# speedup vs baseline: 1.0010x; 1.0010x over previous
"""Trainium2 Bass kernel for AttentionDenseBlock (SE gate + offset conv + deform conv + tanh).

Strategy (per core, data-parallel over batch: 1 sample/core on 8 cores):
  - SE gate: spatial mean -> fc1 -> relu -> fc2 -> sigmoid -> channel scale.
  - Offset conv: 9 shifted bf16 matmuls accumulating in PSUM.
  - Deform conv: bilinear sampling with |offset|<1 decomposes EXACTLY into a
    static 3x3-tap stencil per kernel position with data-dependent weights
    wy in {relu(-dy), 1-|dy|, relu(dy)} (x) wx likewise.  Each of the 81
    (k, r, s) terms is:  out += W_k @ (map_t (*) xs_shifted).
  - Perf structure:
    * tap maps stored k-major in DRAM per quarter; ONE batched
      partition-broadcast DMA per (k, quarter).
    * two bf16 copies of the padded input (even/odd column phase) keep every
      DVE modulation multiply 4B-aligned -> 2x_1P mode.
    * Z_KS kernel positions accumulate their 9 taps on DVE (z-tiles) and hit
      the PE once; their DVE chains are interleaved into the per-tap multiply
      stream so the PE never starves on them.
    * preamble (offset conv -> tap maps) for quarter q+1 is issued between
      main-loop quarters so it overlaps; PE stays warm.
  - Epilogue: tanh(psum + b_conv) fused on ACT -> DMA out.
"""

import os
import sys
from contextlib import ExitStack

import numpy as np

sys.path.insert(0, "/opt/trn_rl_repo")

import concourse.bass as bass
import concourse.bacc as bacc
import concourse.mybir as mybir
import concourse.tile as tile
from concourse.masks import make_identity

B, C, O, H, W = 8, 256, 256, 56, 56
KH = KW = 3
K2 = 9
HP, WP = H + 4, W + 4  # zero-padded by 2 for the 5x5 shift range
HW = H * W
QROWS = 14            # rows per quarter
QN = HW // 4          # 784 spatial positions per quarter
NN = QN // 2          # 392 = matmul N-chunk (fits one PSUM bank)
RED = 16              # SE reduction dim

# kernel positions handled via z-accumulation (DVE) instead of per-tap matmuls
Z_KS = (0, 8)
Z_DVE_TAPS = 9   # all z adds on DVE
TAP_KS = tuple(k for k in range(K2) if k not in Z_KS)

F32 = mybir.dt.float32
BF16 = mybir.dt.bfloat16
AF = mybir.ActivationFunctionType
ALU = mybir.AluOpType

LAST_RESULT = None


def _bcast_ap(base, extra_dims):
    """AP reading `base` ([128, N]) with extra broadcast/reshape free dims."""
    return bass.AP(tensor=base.tensor, offset=base.offset,
                   ap=[list(base.ap[0])] + [list(d) for d in extra_dims])


def build():
    nc = bacc.Bacc()
    x_d = nc.dram_tensor("x", (1, C, H, W), F32, kind="ExternalInput")
    woff_d = nc.dram_tensor("w_off", (2 * K2, C, KH, KW), F32, kind="ExternalInput")
    boff_d = nc.dram_tensor("b_off", (2 * K2,), F32, kind="ExternalInput")
    wconv_d = nc.dram_tensor("w_conv", (O, C, KH, KW), F32, kind="ExternalInput")
    bconv_d = nc.dram_tensor("b_conv", (O,), F32, kind="ExternalInput")
    fc1_d = nc.dram_tensor("fc1", (RED, C), F32, kind="ExternalInput")
    fc2_d = nc.dram_tensor("fc2", (C, RED), F32, kind="ExternalInput")
    out_d = nc.dram_tensor("out", (1, O, H, W), F32, kind="ExternalOutput")

    with tile.TileContext(nc) as tc, ExitStack() as ctx:
        singles = ctx.enter_context(tc.tile_pool(name="singles", bufs=1))
        wyxpool = ctx.enter_context(tc.tile_pool(name="wyxpool", bufs=2))
        mpool = ctx.enter_context(tc.tile_pool(name="mpool", bufs=5))
        mgpool = ctx.enter_context(tc.tile_pool(name="mgpool", bufs=6))
        reppool = ctx.enter_context(tc.tile_pool(name="reppool", bufs=3))
        zreppool = ctx.enter_context(tc.tile_pool(name="zreppool", bufs=1))
        zpool = ctx.enter_context(tc.tile_pool(name="zpool", bufs=2))
        outpool = ctx.enter_context(tc.tile_pool(name="outpool", bufs=2))
        mappool = ctx.enter_context(tc.tile_pool(name="mappool", bufs=2))
        dpool = ctx.enter_context(tc.tile_pool(name="dpool", bufs=1, space="DRAM"))
        psum_pre = ctx.enter_context(tc.tile_pool(name="psum_pre", bufs=1, space="PSUM"))
        psum_main = ctx.enter_context(tc.tile_pool(name="psum_main", bufs=1, space="PSUM"))

        # ---- static tiles ----
        xs_bf = singles.tile([128, 2, HP, WP], BF16)     # padded, scaled, even phase
        xs_bf1 = singles.tile([128, 2, HP, WP], BF16)    # odd phase: col c = col c+1
        wT = singles.tile([128, 2, K2, O], BF16)         # [c, cc, k, o]
        # woffT free dim: [0:9]=dy weights, [32:41]=dx weights (aligned blocks)
        woffT = singles.tile([128, 2, K2, 64], BF16)
        fc1T = singles.tile([128, 2, RED], F32)
        fc2T = singles.tile([128, C], F32)
        bconv = singles.tile([128, 2], F32)
        boff = singles.tile([64, 1], F32)
        y_se = singles.tile([128, 2, 1], F32)
        h_se = singles.tile([128, 1], F32)
        s_se = singles.tile([128, 2, 1], F32)
        boffn = singles.tile([64, 1], F32)
        ident = singles.tile([128, 128], BF16)
        wnat2 = singles.tile([128, 2, C * K2], BF16)
        woff_nat2 = singles.tile([2 * K2, C * K2], BF16)
        fc1Tb = singles.tile([128, 2, RED], F32)
        fc2Tb = singles.tile([128, C], F32)
        # k-major tap maps per quarter: row k*9 + (3r+s)
        maps_dram = [dpool.tile([K2 * K2, QN], BF16, name=f"maps{q}") for q in range(4)]

        make_identity(nc, ident[:, :])

        # ---- input DMA + weight DMA (cast to bf16 during DMA on SWDGE) ----
        nc.vector.memset(xs_bf[:, :, :, :], 0.0)
        x_r = x_d[:].rearrange("one c h w -> (one c) h w")
        for cc in range(2):
            nc.gpsimd.dma_start(out=xs_bf[:, cc, 2:2 + H, 2:2 + W],
                                in_=x_r[cc * 128:(cc + 1) * 128, :, :])
        wc_r = wconv_d[:].rearrange("o c kh kw -> o (c kh kw)")
        for oc in range(2):
            nc.gpsimd.dma_start(out=wnat2[:, oc, :], in_=wc_r[oc * 128:(oc + 1) * 128, :])
        nc.gpsimd.dma_start(out=woff_nat2[:, :],
                            in_=woff_d[:].rearrange("o c kh kw -> o (c kh kw)"))
        fc1_r = fc1_d[:].rearrange("m c -> c m")
        for cc in range(2):
            nc.sync.dma_start(out=fc1T[:, cc, :], in_=fc1_r[cc * 128:(cc + 1) * 128, :])
        nc.vector.memset(fc2T[:, :], 0.0)
        nc.sync.dma_start(out=fc2T[0:RED, :], in_=fc2_d[:].rearrange("c m -> m c"))
        nc.sync.dma_start(out=bconv[:, :],
                          in_=bconv_d[:].rearrange("(a c) -> c a", a=2))
        # b_off loaded de-interleaved: dy biases -> rows 0:9, dx -> rows 32:41
        nc.vector.memset(boff[:, :], 0.0)
        boff_src = boff_d[:]
        nc.sync.dma_start(out=boff[0:K2, 0:1],
                          in_=bass.AP(tensor=boff_src.tensor, offset=boff_src.offset,
                                      ap=[[2, K2], [0, 1]]))
        nc.sync.dma_start(out=boff[32:32 + K2, 0:1],
                          in_=bass.AP(tensor=boff_src.tensor,
                                      offset=boff_src.offset + 1,
                                      ap=[[2, K2], [0, 1]]))

        # ---- SE gate (mean over unscaled x, then scale xs in place) ----
        for cc in range(2):
            nc.vector.tensor_reduce(out=y_se[:, cc, 0:1], in_=xs_bf[:, cc, :, :],
                                    axis=mybir.AxisListType.XY, op=ALU.add)
        nc.vector.tensor_scalar_mul(y_se[:, :, 0:1], y_se[:, :, 0:1], 1.0 / HW)
        nc.vector.tensor_copy(fc1Tb[:, :, :], fc1T[:, :, :])
        nc.vector.tensor_copy(fc2Tb[:, :], fc2T[:, :])
        h_ps = psum_pre.tile([128, RED], F32, tag="se")
        for cc in range(2):
            nc.tensor.matmul(h_ps[0:RED, 0:1], lhsT=fc1Tb[:, cc, :], rhs=y_se[:, cc, 0:1],
                             start=(cc == 0), stop=(cc == 1))
        nc.vector.memset(h_se[:, :], 0.0)
        nc.vector.tensor_relu(h_se[0:RED, 0:1], h_ps[0:RED, 0:1])
        for cc in range(2):
            s_ps = psum_pre.tile([128, RED], F32, tag="se")
            nc.tensor.matmul(s_ps[:, 0:1], lhsT=fc2Tb[:, cc * 128:(cc + 1) * 128],
                             rhs=h_se[:, 0:1], start=True, stop=True)
            nc.scalar.activation(s_se[:, cc, 0:1], s_ps[:, 0:1], AF.Sigmoid)
        for cc in range(2):
            nc.vector.tensor_scalar_mul(xs_bf[:, cc, :, :], xs_bf[:, cc, :, :],
                                        s_se[:, cc, 0:1])
        # odd column phase (cols 0..57 used by tap windows; col 58/59 never read)
        nc.vector.tensor_copy(xs_bf1[:, :, :, 0:WP - 1], xs_bf[:, :, :, 1:WP])

        nc.vector.memset(woffT[:, :, :, :], 0.0)

        # ---- transpose conv weights on PE: wT[c, cc, k, o] ----
        for kk in range(K2):
            for cc in range(2):
                for oc in range(2):
                    tp = psum_pre.tile([128, 128], BF16, tag="tp")
                    src = wnat2[:, oc, :].rearrange("p (c k) -> p c k", k=K2)
                    nc.tensor.transpose(tp[:, :], src[:, cc * 128:(cc + 1) * 128, kk],
                                        ident[:, :])
                    nc.vector.tensor_copy(wT[:, cc, kk, oc * 128:(oc + 1) * 128],
                                          tp[:, :])
                tp = psum_pre.tile([128, 128], BF16, tag="tp")
                srco = woff_nat2[:, :].rearrange("p (c k) -> p c k", k=K2)
                nc.tensor.transpose(tp[:, 0:2 * K2],
                                    srco[:, cc * 128:(cc + 1) * 128, kk],
                                    ident[0:2 * K2, 0:2 * K2])
                # de-interleave offset channels: dy -> cols 0:9, dx -> cols 32:41
                nc.vector.tensor_copy(woffT[:, cc, kk, 0:K2], tp[:, 0:2 * K2:2])
                nc.vector.tensor_copy(woffT[:, cc, kk, 32:32 + K2],
                                      tp[:, 1:2 * K2:2])

        nc.scalar.activation(boffn[:, 0:1], boff[:, 0:1], AF.Copy, scale=-1.0)

        # ---- per-quarter preamble: offset conv -> tap-weight maps -> DRAM ----
        def preamble(q):
            wy0 = wyxpool.tile([K2, QN], BF16, tag="wy0")
            wy1 = wyxpool.tile([K2, QN], BF16, tag="wy1")
            wy2 = wyxpool.tile([K2, QN], BF16, tag="wy2")
            wx0 = wyxpool.tile([K2, QN], BF16, tag="wx0")
            wx1 = wyxpool.tile([K2, QN], BF16, tag="wx1")
            wx2 = wyxpool.tile([K2, QN], BF16, tag="wx2")
            for nn in range(2):
                off_ps = psum_pre.tile([64, NN], F32, tag="off")
                for kk in range(K2):
                    ki, kj = divmod(kk, 3)
                    dh, dw = ki - 1, kj - 1
                    for cc in range(2):
                        r0 = 2 + dh + q * QROWS + nn * (QROWS // 2)
                        rhs = xs_bf[:, cc, r0:r0 + QROWS // 2, 2 + dw:2 + dw + W]
                        nc.tensor.matmul(off_ps[0:64, :],
                                         lhsT=woffT[:, cc, kk, 0:64], rhs=rhs,
                                         start=(kk == 0 and cc == 0),
                                         stop=(kk == K2 - 1 and cc == 1))
                # offset = psum + b_off, fused into relu(+-offset) tap weights
                nsl = slice(nn * NN, (nn + 1) * NN)
                nc.scalar.activation(wy0[:, nsl], off_ps[0:K2, :], AF.Relu,
                                     scale=-1.0, bias=boffn[0:K2, 0:1])
                nc.scalar.activation(wy2[:, nsl], off_ps[0:K2, :], AF.Relu,
                                     scale=1.0, bias=boff[0:K2, 0:1])
                nc.scalar.activation(wx0[:, nsl], off_ps[32:32 + K2, :], AF.Relu,
                                     scale=-1.0, bias=boffn[32:32 + K2, 0:1])
                nc.scalar.activation(wx2[:, nsl], off_ps[32:32 + K2, :], AF.Relu,
                                     scale=1.0, bias=boff[32:32 + K2, 0:1])
            # wy1 = 1 - |dy| = 1 - (relu(dy) + relu(-dy)); same for wx1
            nc.vector.tensor_add(wy1[:, :], wy0[:, :], wy2[:, :])
            nc.scalar.activation(wy1[:, :], wy1[:, :], AF.Copy, scale=-1.0, bias=1.0)
            nc.vector.tensor_add(wx1[:, :], wx0[:, :], wx2[:, :])
            nc.scalar.activation(wx1[:, :], wx1[:, :], AF.Copy, scale=-1.0, bias=1.0)
            wys = (wy0, wy1, wy2)
            wxs = (wx0, wx1, wx2)
            md = maps_dram[q][0:1, 0:1]

            def make_prod(r, s):
                def op():
                    rs = 3 * r + s
                    mtmp = mappool.tile([K2, QN], BF16, name="mtmp")
                    nc.vector.tensor_mul(mtmp[:, :], wys[r][:, :], wxs[s][:, :])
                    nc.sync.dma_start(
                        out=bass.AP(tensor=md.tensor, offset=md.offset + rs * QN,
                                    ap=[[K2 * QN, K2], [1, QN]]),
                        in_=mtmp[:, :])
                return op
            return [make_prod(r, s) for r in range(3) for s in range(3)]

        # ---- main deform-conv loop for one quarter ----
        n_events = len(TAP_KS) * K2 + len(Z_KS)

        def mod_mul(out_ap, mrep_row, kk, r, s, q, engine=None):
            ki, kj = divmod(kk, 3)
            dh, dw = ki - 1 + r - 1, kj - 1 + s - 1
            r0 = 2 + dh + q * QROWS
            cs = 2 + dw
            if cs % 2 == 0:
                xs_win = xs_bf[:, :, r0:r0 + QROWS, cs:cs + W]
            else:
                xs_win = xs_bf1[:, :, r0:r0 + QROWS, cs - 1:cs - 1 + W]
            mrep_b = _bcast_ap(mrep_row, [[0, 2], [W, QROWS], [1, W]])
            (engine or nc.vector).tensor_tensor(out_ap, xs_win, mrep_b, op=ALU.mult)

        def main_quarter(q, extra_ops=()):
            ps = [psum_main.tile([128, QN], F32, tag=f"ps{oc}", name=f"ps{oc}")
                  for oc in range(2)]

            def do_mms(rhs_tile, kk, ev):
                for cc in range(2):
                    for oc in range(2):
                        for n0, n1 in ((0, 512), (512, QN)):
                            nc.tensor.matmul(
                                ps[oc][:, n0:n1],
                                lhsT=wT[:, cc, kk, oc * 128:(oc + 1) * 128],
                                rhs=rhs_tile[:, cc, n0:n1],
                                start=(ev == 0 and cc == 0),
                                stop=(ev == n_events - 1 and cc == 1))

            md = maps_dram[q][0:1, 0:1]

            def bcast(out_ap, kk):
                nc.gpsimd.dma_start(
                    out=out_ap,
                    in_=bass.AP(tensor=md.tensor, offset=md.offset + kk * K2 * QN,
                                ap=[[0, 128], [QN, K2], [1, QN]]))

            # z-map broadcasts + per-tap k lookahead broadcast queue
            zrep = zreppool.tile([128, len(Z_KS), K2, QN], BF16, name="zrep")
            for zi, zk in enumerate(Z_KS):
                bcast(zrep[:, zi, :, :], zk)
            reps = {}
            reps[TAP_KS[0]] = reppool.tile([128, K2, QN], BF16, name="rep")
            bcast(reps[TAP_KS[0]][:, :, :], TAP_KS[0])

            # build deferred z-op list: [(is_first_mult?, zi, zk, r, s)]
            zaccs = {}
            for zi, zk in enumerate(Z_KS):
                zaccs[zi] = zpool.tile([128, 2, QN], BF16, name=f"zacc{zi}")
            zops = []
            for t in range(K2):
                for zi, zk in enumerate(Z_KS):
                    zops.append((zi, zk, t))

            def issue_zop(idx):
                zi, zk, t = zops[idx]
                r, s = divmod(t, 3)
                acc = zaccs[zi]
                if t == 0:
                    acc_v = acc[:, :, :].rearrange("p a (r c) -> p a r c", c=W)
                    mod_mul(acc_v, zrep[:, zi, t, :], zk, r, s, q)
                elif t < Z_DVE_TAPS:
                    m = mpool.tile([128, 2, QN], BF16)
                    m_v = m[:, :, :].rearrange("p a (r c) -> p a r c", c=W)
                    mod_mul(m_v, zrep[:, zi, t, :], zk, r, s, q)
                    nc.vector.tensor_add(acc[:, :, :], acc[:, :, :], m[:, :, :])
                else:
                    m = mgpool.tile([128, 2, QN], BF16, name="mg")
                    m_v = m[:, :, :].rearrange("p a (r c) -> p a r c", c=W)
                    mod_mul(m_v, zrep[:, zi, t, :], zk, r, s, q)
                    nc.vector.tensor_add(acc[:, :, :], acc[:, :, :], m[:, :, :])

            # interleave: hold z-ops for the first Z_DELAY taps (zrep DMA still
            # in flight), then catch up evenly so chains finish ~5 taps early
            Z_DELAY = 8
            n_taps = len(TAP_KS) * K2
            zdone = 0
            ev = 0
            tap_i = 0
            for ki, kk in enumerate(TAP_KS):
                if ki + 1 < len(TAP_KS):
                    nk = TAP_KS[ki + 1]
                    reps[nk] = reppool.tile([128, K2, QN], BF16, name="rep")
                    bcast(reps[nk][:, :, :], nk)
                for t in range(K2):
                    r, s = divmod(t, 3)
                    m = mpool.tile([128, 2, QN], BF16)
                    m_v = m[:, :, :].rearrange("p a (r c) -> p a r c", c=W)
                    mod_mul(m_v, reps[kk][:, t, :], kk, r, s, q)
                    do_mms(m, kk, ev)
                    ev += 1
                    tap_i += 1
                    # sprinkle next-next quarter's map products into the stream
                    if extra_ops and tap_i % 4 == 0 and tap_i // 4 <= len(extra_ops):
                        extra_ops[tap_i // 4 - 1]()
                    if tap_i > Z_DELAY:
                        ztarget = min(len(zops),
                                      (len(zops) * (tap_i - Z_DELAY)) // (n_taps - Z_DELAY - 18) if n_taps - Z_DELAY - 18 > 0 else len(zops))
                        while zdone < ztarget:
                            issue_zop(zdone)
                            zdone += 1
            while zdone < len(zops):
                issue_zop(zdone)
                zdone += 1
            for zi, zk in enumerate(Z_KS):
                do_mms(zaccs[zi], zk, ev)
                ev += 1
            assert ev == n_events

            out_r = out_d[:].rearrange("one o h w -> (one o) h w")
            for oc in range(2):
                osb = outpool.tile([128, QN], F32)
                nc.scalar.activation(osb[:, :], ps[oc][:, :], AF.Tanh,
                                     bias=bconv[:, oc:oc + 1])
                nc.sync.dma_start(
                    out=out_r[oc * 128:(oc + 1) * 128,
                              q * QROWS:(q + 1) * QROWS, :],
                    in_=osb[:, :])

        # software pipeline: preamble(q+1)'s PE/ACT part issued between main
        # quarters; its DVE product ops injected into main(q)'s tap stream
        prods0 = preamble(0)
        for op in prods0:
            op()
        prods1 = preamble(1)
        prods2 = preamble(2)
        main_quarter(0, prods1)
        prods3 = preamble(3)
        main_quarter(1, prods2)
        main_quarter(2, prods3)
        main_quarter(3)
    nc.finalize()
    return nc


_NC = None


def _get_nc():
    global _NC
    if _NC is None:
        _NC = build()
    return _NC


def kernel(**inputs):
    global LAST_RESULT
    from concourse.bass_utils import run_bass_kernel_spmd

    nc = _get_nc()
    x = np.ascontiguousarray(inputs["x"], dtype=np.float32)
    shared = {k: np.ascontiguousarray(np.asarray(inputs[k]), dtype=np.float32)
              for k in ("w_off", "b_off", "w_conv", "b_conv", "fc1", "fc2")}
    in_maps = [{"x": x[i:i + 1], **shared} for i in range(B)]
    res = run_bass_kernel_spmd(nc, in_maps, core_ids=list(range(B)),
                               trace=bool(int(os.environ.get("KB_TRACE", "0"))))
    LAST_RESULT = res
    out = np.concatenate([res.results[i]["out"] for i in range(B)], axis=0)
    return out.astype(np.float32)


if __name__ == "__main__":
    nc = build()
    print("build OK")


# revision 11
# speedup vs baseline: 1.0541x; 1.0530x over previous
"""Trainium2 Bass kernel for AttentionDenseBlock (SE gate + offset conv + deform conv + tanh).

Strategy (per core, data-parallel over batch: 1 sample/core on 8 cores):
  - SE gate: spatial mean -> fc1 -> relu -> fc2 -> sigmoid -> channel scale.
  - Offset conv: 9 shifted bf16 matmuls accumulating in PSUM.
  - Deform conv: bilinear sampling with |offset|<1 decomposes EXACTLY into a
    static 3x3-tap stencil per kernel position with data-dependent weights
    wy in {relu(-dy), 1-|dy|, relu(dy)} (x) wx likewise.  Each of the 81
    (k, r, s) terms is:  out += W_k @ (map_t (*) xs_shifted).
  - Perf structure:
    * tap maps stored k-major in DRAM per quarter; ONE batched
      partition-broadcast DMA per (k, quarter).
    * two bf16 copies of the padded input (even/odd column phase) keep every
      DVE modulation multiply 4B-aligned -> 2x_1P mode.
    * Z_KS kernel positions accumulate their 9 taps on DVE (z-tiles) and hit
      the PE once; their DVE chains are interleaved into the per-tap multiply
      stream so the PE never starves on them.
    * preamble (offset conv -> tap maps) for quarter q+1 is issued between
      main-loop quarters so it overlaps; PE stays warm.
  - Epilogue: tanh(psum + b_conv) fused on ACT -> DMA out.
"""

import os
import sys
from contextlib import ExitStack

import numpy as np

sys.path.insert(0, "/opt/trn_rl_repo")

import concourse.bass as bass
import concourse.bacc as bacc
import concourse.mybir as mybir
import concourse.tile as tile
from concourse.masks import make_identity

B, C, O, H, W = 8, 256, 256, 56, 56
KH = KW = 3
K2 = 9
HP, WP = H + 4, W + 4  # zero-padded by 2 for the 5x5 shift range
HW = H * W
QROWS = 14            # rows per quarter
QN = HW // 4          # 784 spatial positions per quarter
NN = QN // 2          # 392 = matmul N-chunk (fits one PSUM bank)
RED = 16              # SE reduction dim

# kernel positions handled via z-accumulation (DVE) instead of per-tap matmuls
Z_KS = (0, 8)
Z_DVE_TAPS = 9   # all z adds on DVE
TAP_KS = tuple(k for k in range(K2) if k not in Z_KS)

F32 = mybir.dt.float32
BF16 = mybir.dt.bfloat16
AF = mybir.ActivationFunctionType
ALU = mybir.AluOpType

LAST_RESULT = None


def _bcast_ap(base, extra_dims):
    """AP reading `base` ([128, N]) with extra broadcast/reshape free dims."""
    return bass.AP(tensor=base.tensor, offset=base.offset,
                   ap=[list(base.ap[0])] + [list(d) for d in extra_dims])


def build():
    nc = bacc.Bacc()
    x_d = nc.dram_tensor("x", (1, C, H, W), F32, kind="ExternalInput")
    woff_d = nc.dram_tensor("w_off", (2 * K2, C, KH, KW), F32, kind="ExternalInput")
    boff_d = nc.dram_tensor("b_off", (2 * K2,), F32, kind="ExternalInput")
    wconv_d = nc.dram_tensor("w_conv", (O, C, KH, KW), F32, kind="ExternalInput")
    bconv_d = nc.dram_tensor("b_conv", (O,), F32, kind="ExternalInput")
    fc1_d = nc.dram_tensor("fc1", (RED, C), F32, kind="ExternalInput")
    fc2_d = nc.dram_tensor("fc2", (C, RED), F32, kind="ExternalInput")
    out_d = nc.dram_tensor("out", (1, O, H, W), F32, kind="ExternalOutput")

    with tile.TileContext(nc) as tc, ExitStack() as ctx:
        singles = ctx.enter_context(tc.tile_pool(name="singles", bufs=1))
        wyxpool = ctx.enter_context(tc.tile_pool(name="wyxpool", bufs=2))
        mpool = ctx.enter_context(tc.tile_pool(name="mpool", bufs=8))
        mgpool = ctx.enter_context(tc.tile_pool(name="mgpool", bufs=6))
        reppool = ctx.enter_context(tc.tile_pool(name="reppool", bufs=2))
        zreppool = ctx.enter_context(tc.tile_pool(name="zreppool", bufs=1))
        zpool = ctx.enter_context(tc.tile_pool(name="zpool", bufs=2))
        outpool = ctx.enter_context(tc.tile_pool(name="outpool", bufs=2))
        mappool = ctx.enter_context(tc.tile_pool(name="mappool", bufs=2))
        dpool = ctx.enter_context(tc.tile_pool(name="dpool", bufs=1, space="DRAM"))
        psum_pre = ctx.enter_context(tc.tile_pool(name="psum_pre", bufs=1, space="PSUM"))
        psum_main = ctx.enter_context(tc.tile_pool(name="psum_main", bufs=1, space="PSUM"))

        # ---- static tiles ----
        xs_bf = singles.tile([128, 2, HP, WP], BF16)     # padded, scaled, even phase
        xs_bf1 = singles.tile([128, 2, HP, WP], BF16)    # odd phase: col c = col c+1
        wT = singles.tile([128, 2, K2, O], BF16)         # [c, cc, k, o]
        # woffT free dim: [0:9]=dy weights, [32:41]=dx weights (aligned blocks)
        woffT = singles.tile([128, 2, K2, 64], BF16)
        fc1T = singles.tile([128, 2, RED], F32)
        fc2T = singles.tile([128, C], F32)
        bconv = singles.tile([128, 2], F32)
        boff = singles.tile([64, 1], F32)
        y_se = singles.tile([128, 2, 1], F32)
        h_se = singles.tile([128, 1], F32)
        s_se = singles.tile([128, 2, 1], F32)
        boffn = singles.tile([64, 1], F32)
        ident = singles.tile([128, 128], BF16)
        wnat2 = singles.tile([128, 2, C * K2], BF16)
        woff_nat2 = singles.tile([2 * K2, C * K2], BF16)
        fc1Tb = singles.tile([128, 2, RED], F32)
        fc2Tb = singles.tile([128, C], F32)
        # k-major tap maps per quarter: row k*9 + (3r+s)
        maps_dram = [dpool.tile([K2 * K2, QN], BF16, name=f"maps{q}") for q in range(4)]

        make_identity(nc, ident[:, :])

        # ---- input DMA + weight DMA (cast to bf16 during DMA on SWDGE) ----
        nc.vector.memset(xs_bf[:, :, :, :], 0.0)
        x_r = x_d[:].rearrange("one c h w -> (one c) h w")
        for cc in range(2):
            nc.gpsimd.dma_start(out=xs_bf[:, cc, 2:2 + H, 2:2 + W],
                                in_=x_r[cc * 128:(cc + 1) * 128, :, :])
        wc_r = wconv_d[:].rearrange("o c kh kw -> o (c kh kw)")
        for oc in range(2):
            nc.gpsimd.dma_start(out=wnat2[:, oc, :], in_=wc_r[oc * 128:(oc + 1) * 128, :])
        nc.gpsimd.dma_start(out=woff_nat2[:, :],
                            in_=woff_d[:].rearrange("o c kh kw -> o (c kh kw)"))
        fc1_r = fc1_d[:].rearrange("m c -> c m")
        for cc in range(2):
            nc.sync.dma_start(out=fc1T[:, cc, :], in_=fc1_r[cc * 128:(cc + 1) * 128, :])
        nc.vector.memset(fc2T[:, :], 0.0)
        nc.sync.dma_start(out=fc2T[0:RED, :], in_=fc2_d[:].rearrange("c m -> m c"))
        nc.sync.dma_start(out=bconv[:, :],
                          in_=bconv_d[:].rearrange("(a c) -> c a", a=2))
        # b_off loaded de-interleaved: dy biases -> rows 0:9, dx -> rows 32:41
        nc.vector.memset(boff[:, :], 0.0)
        boff_src = boff_d[:]
        nc.sync.dma_start(out=boff[0:K2, 0:1],
                          in_=bass.AP(tensor=boff_src.tensor, offset=boff_src.offset,
                                      ap=[[2, K2], [0, 1]]))
        nc.sync.dma_start(out=boff[32:32 + K2, 0:1],
                          in_=bass.AP(tensor=boff_src.tensor,
                                      offset=boff_src.offset + 1,
                                      ap=[[2, K2], [0, 1]]))

        # ---- SE gate (mean over unscaled x, then scale xs in place) ----
        for cc in range(2):
            nc.vector.tensor_reduce(out=y_se[:, cc, 0:1], in_=xs_bf[:, cc, :, :],
                                    axis=mybir.AxisListType.XY, op=ALU.add)
        nc.vector.tensor_scalar_mul(y_se[:, :, 0:1], y_se[:, :, 0:1], 1.0 / HW)
        nc.vector.tensor_copy(fc1Tb[:, :, :], fc1T[:, :, :])
        nc.vector.tensor_copy(fc2Tb[:, :], fc2T[:, :])
        h_ps = psum_pre.tile([128, RED], F32, tag="se")
        for cc in range(2):
            nc.tensor.matmul(h_ps[0:RED, 0:1], lhsT=fc1Tb[:, cc, :], rhs=y_se[:, cc, 0:1],
                             start=(cc == 0), stop=(cc == 1))
        nc.vector.memset(h_se[:, :], 0.0)
        nc.vector.tensor_relu(h_se[0:RED, 0:1], h_ps[0:RED, 0:1])
        for cc in range(2):
            s_ps = psum_pre.tile([128, RED], F32, tag="se")
            nc.tensor.matmul(s_ps[:, 0:1], lhsT=fc2Tb[:, cc * 128:(cc + 1) * 128],
                             rhs=h_se[:, 0:1], start=True, stop=True)
            nc.scalar.activation(s_se[:, cc, 0:1], s_ps[:, 0:1], AF.Sigmoid)
        for cc in range(2):
            nc.vector.tensor_scalar_mul(xs_bf[:, cc, :, :], xs_bf[:, cc, :, :],
                                        s_se[:, cc, 0:1])
        # odd column phase (cols 0..57 used by tap windows; col 58/59 never read)
        nc.vector.tensor_copy(xs_bf1[:, :, :, 0:WP - 1], xs_bf[:, :, :, 1:WP])

        nc.vector.memset(woffT[:, :, :, :], 0.0)

        # ---- transpose conv weights on PE: wT[c, cc, k, o] ----
        for kk in range(K2):
            for cc in range(2):
                for oc in range(2):
                    tp = psum_pre.tile([128, 128], BF16, tag="tp")
                    src = wnat2[:, oc, :].rearrange("p (c k) -> p c k", k=K2)
                    nc.tensor.transpose(tp[:, :], src[:, cc * 128:(cc + 1) * 128, kk],
                                        ident[:, :])
                    nc.vector.tensor_copy(wT[:, cc, kk, oc * 128:(oc + 1) * 128],
                                          tp[:, :])
                tp = psum_pre.tile([128, 128], BF16, tag="tp")
                srco = woff_nat2[:, :].rearrange("p (c k) -> p c k", k=K2)
                nc.tensor.transpose(tp[:, 0:2 * K2],
                                    srco[:, cc * 128:(cc + 1) * 128, kk],
                                    ident[0:2 * K2, 0:2 * K2])
                # de-interleave offset channels: dy -> cols 0:9, dx -> cols 32:41
                nc.vector.tensor_copy(woffT[:, cc, kk, 0:K2], tp[:, 0:2 * K2:2])
                nc.vector.tensor_copy(woffT[:, cc, kk, 32:32 + K2],
                                      tp[:, 1:2 * K2:2])

        nc.scalar.activation(boffn[:, 0:1], boff[:, 0:1], AF.Copy, scale=-1.0)

        # ---- per-quarter preamble: offset conv -> tap-weight maps -> DRAM ----
        def preamble(q):
            wy0 = wyxpool.tile([K2, QN], BF16, tag="wy0")
            wy1 = wyxpool.tile([K2, QN], BF16, tag="wy1")
            wy2 = wyxpool.tile([K2, QN], BF16, tag="wy2")
            wx0 = wyxpool.tile([K2, QN], BF16, tag="wx0")
            wx1 = wyxpool.tile([K2, QN], BF16, tag="wx1")
            wx2 = wyxpool.tile([K2, QN], BF16, tag="wx2")
            for nn in range(2):
                off_ps = psum_pre.tile([64, NN], F32, tag="off")
                for kk in range(K2):
                    ki, kj = divmod(kk, 3)
                    dh, dw = ki - 1, kj - 1
                    for cc in range(2):
                        r0 = 2 + dh + q * QROWS + nn * (QROWS // 2)
                        rhs = xs_bf[:, cc, r0:r0 + QROWS // 2, 2 + dw:2 + dw + W]
                        nc.tensor.matmul(off_ps[0:64, :],
                                         lhsT=woffT[:, cc, kk, 0:64], rhs=rhs,
                                         start=(kk == 0 and cc == 0),
                                         stop=(kk == K2 - 1 and cc == 1))
                # offset = psum + b_off, fused into relu(+-offset) tap weights
                nsl = slice(nn * NN, (nn + 1) * NN)
                nc.scalar.activation(wy0[:, nsl], off_ps[0:K2, :], AF.Relu,
                                     scale=-1.0, bias=boffn[0:K2, 0:1])
                nc.scalar.activation(wy2[:, nsl], off_ps[0:K2, :], AF.Relu,
                                     scale=1.0, bias=boff[0:K2, 0:1])
                nc.scalar.activation(wx0[:, nsl], off_ps[32:32 + K2, :], AF.Relu,
                                     scale=-1.0, bias=boffn[32:32 + K2, 0:1])
                nc.scalar.activation(wx2[:, nsl], off_ps[32:32 + K2, :], AF.Relu,
                                     scale=1.0, bias=boff[32:32 + K2, 0:1])
            # wy1 = 1 - |dy| = 1 - (relu(dy) + relu(-dy)); same for wx1
            nc.vector.tensor_add(wy1[:, :], wy0[:, :], wy2[:, :])
            nc.scalar.activation(wy1[:, :], wy1[:, :], AF.Copy, scale=-1.0, bias=1.0)
            nc.vector.tensor_add(wx1[:, :], wx0[:, :], wx2[:, :])
            nc.scalar.activation(wx1[:, :], wx1[:, :], AF.Copy, scale=-1.0, bias=1.0)
            wys = (wy0, wy1, wy2)
            wxs = (wx0, wx1, wx2)
            md = maps_dram[q][0:1, 0:1]

            def make_prod(r, s):
                def op():
                    rs = 3 * r + s
                    mtmp = mappool.tile([K2, QN], BF16, name="mtmp")
                    nc.vector.tensor_mul(mtmp[:, :], wys[r][:, :], wxs[s][:, :])
                    nc.sync.dma_start(
                        out=bass.AP(tensor=md.tensor, offset=md.offset + rs * QN,
                                    ap=[[K2 * QN, K2], [1, QN]]),
                        in_=mtmp[:, :])
                return op
            return [make_prod(r, s) for r in range(3) for s in range(3)]

        # ---- main deform-conv loop for one quarter ----
        n_events = len(TAP_KS) * K2 + len(Z_KS)

        def mod_mul(out_ap, mrep_row, kk, r, s, q, engine=None):
            ki, kj = divmod(kk, 3)
            dh, dw = ki - 1 + r - 1, kj - 1 + s - 1
            r0 = 2 + dh + q * QROWS
            cs = 2 + dw
            if cs % 2 == 0:
                xs_win = xs_bf[:, :, r0:r0 + QROWS, cs:cs + W]
            else:
                xs_win = xs_bf1[:, :, r0:r0 + QROWS, cs - 1:cs - 1 + W]
            mrep_b = _bcast_ap(mrep_row, [[0, 2], [W, QROWS], [1, W]])
            (engine or nc.vector).tensor_tensor(out_ap, xs_win, mrep_b, op=ALU.mult)

        def main_quarter(q, extra_ops=()):
            ps = [psum_main.tile([128, QN], F32, tag=f"ps{oc}", name=f"ps{oc}")
                  for oc in range(2)]

            def do_mms(rhs_tile, kk, ev):
                for cc in range(2):
                    for oc in range(2):
                        for n0, n1 in ((0, 512), (512, QN)):
                            nc.tensor.matmul(
                                ps[oc][:, n0:n1],
                                lhsT=wT[:, cc, kk, oc * 128:(oc + 1) * 128],
                                rhs=rhs_tile[:, cc, n0:n1],
                                start=(ev == 0 and cc == 0),
                                stop=(ev == n_events - 1 and cc == 1))

            md = maps_dram[q][0:1, 0:1]

            def bcast(out_ap, kk):
                nc.gpsimd.dma_start(
                    out=out_ap,
                    in_=bass.AP(tensor=md.tensor, offset=md.offset + kk * K2 * QN,
                                ap=[[0, 128], [QN, K2], [1, QN]]))

            # z-map broadcasts + per-tap k lookahead broadcast queue
            zrep = zreppool.tile([128, len(Z_KS), K2, QN], BF16, name="zrep")
            for zi, zk in enumerate(Z_KS):
                bcast(zrep[:, zi, :, :], zk)
            reps = {}
            reps[TAP_KS[0]] = reppool.tile([128, K2, QN], BF16, name="rep")
            bcast(reps[TAP_KS[0]][:, :, :], TAP_KS[0])

            # build deferred z-op list: [(is_first_mult?, zi, zk, r, s)]
            zaccs = {}
            for zi, zk in enumerate(Z_KS):
                zaccs[zi] = zpool.tile([128, 2, QN], BF16, name=f"zacc{zi}")
            zops = []
            for zi, zk in enumerate(Z_KS):
                for t in range(K2):
                    zops.append((zi, zk, t))

            def issue_zop(idx):
                zi, zk, t = zops[idx]
                r, s = divmod(t, 3)
                acc = zaccs[zi]
                if t == 0:
                    acc_v = acc[:, :, :].rearrange("p a (r c) -> p a r c", c=W)
                    mod_mul(acc_v, zrep[:, zi, t, :], zk, r, s, q)
                elif t < Z_DVE_TAPS:
                    m = mpool.tile([128, 2, QN], BF16)
                    m_v = m[:, :, :].rearrange("p a (r c) -> p a r c", c=W)
                    mod_mul(m_v, zrep[:, zi, t, :], zk, r, s, q)
                    nc.vector.tensor_add(acc[:, :, :], acc[:, :, :], m[:, :, :])
                else:
                    m = mgpool.tile([128, 2, QN], BF16, name="mg")
                    m_v = m[:, :, :].rearrange("p a (r c) -> p a r c", c=W)
                    mod_mul(m_v, zrep[:, zi, t, :], zk, r, s, q)
                    nc.vector.tensor_add(acc[:, :, :], acc[:, :, :], m[:, :, :])

            # interleave: hold z-ops for the first Z_DELAY taps (zrep DMA still
            # in flight), then catch up evenly so chains finish ~5 taps early
            Z_DELAY = 10
            n_taps = len(TAP_KS) * K2
            zdone = 0
            ev = 0
            tap_i = 0
            for ki, kk in enumerate(TAP_KS):
                if ki + 1 < len(TAP_KS):
                    nk = TAP_KS[ki + 1]
                    reps[nk] = reppool.tile([128, K2, QN], BF16, name="rep")
                    bcast(reps[nk][:, :, :], nk)
                for t in range(K2):
                    r, s = divmod(t, 3)
                    m = mpool.tile([128, 2, QN], BF16)
                    m_v = m[:, :, :].rearrange("p a (r c) -> p a r c", c=W)
                    mod_mul(m_v, reps[kk][:, t, :], kk, r, s, q)
                    do_mms(m, kk, ev)
                    ev += 1
                    tap_i += 1
                    if tap_i > Z_DELAY:
                        ztarget = min(len(zops),
                                      (len(zops) * (tap_i - Z_DELAY)) // (n_taps - Z_DELAY - 5) if n_taps - Z_DELAY - 5 > 0 else len(zops))
                        while zdone < ztarget:
                            issue_zop(zdone)
                            zdone += 1
            while zdone < len(zops):
                issue_zop(zdone)
                zdone += 1
            for zi, zk in enumerate(Z_KS):
                do_mms(zaccs[zi], zk, ev)
                ev += 1
            assert ev == n_events

            out_r = out_d[:].rearrange("one o h w -> (one o) h w")
            for oc in range(2):
                osb = outpool.tile([128, QN], F32)
                nc.scalar.activation(osb[:, :], ps[oc][:, :], AF.Tanh,
                                     bias=bconv[:, oc:oc + 1])
                nc.sync.dma_start(
                    out=out_r[oc * 128:(oc + 1) * 128,
                              q * QROWS:(q + 1) * QROWS, :],
                    in_=osb[:, :])

        # software pipeline: preamble(q+1) issued between main quarters
        prods = preamble(0)
        for op in prods:
            op()
        prods = preamble(1)
        for op in prods:
            op()
        main_quarter(0)
        prods = preamble(2)
        for op in prods:
            op()
        main_quarter(1)
        prods = preamble(3)
        for op in prods:
            op()
        main_quarter(2)
        main_quarter(3)
    nc.finalize()
    return nc


_NC = None


def _get_nc():
    global _NC
    if _NC is None:
        _NC = build()
    return _NC


def kernel(**inputs):
    global LAST_RESULT
    from concourse.bass_utils import run_bass_kernel_spmd

    nc = _get_nc()
    x = np.ascontiguousarray(inputs["x"], dtype=np.float32)
    shared = {k: np.ascontiguousarray(np.asarray(inputs[k]), dtype=np.float32)
              for k in ("w_off", "b_off", "w_conv", "b_conv", "fc1", "fc2")}
    in_maps = [{"x": x[i:i + 1], **shared} for i in range(B)]
    res = run_bass_kernel_spmd(nc, in_maps, core_ids=list(range(B)),
                               trace=bool(int(os.environ.get("KB_TRACE", "0"))))
    LAST_RESULT = res
    out = np.concatenate([res.results[i]["out"] for i in range(B)], axis=0)
    return out.astype(np.float32)


if __name__ == "__main__":
    nc = build()
    print("build OK")


# revision 12
# speedup vs baseline: 1.0670x; 1.0123x over previous
"""Trainium2 Bass kernel for AttentionDenseBlock (SE gate + offset conv + deform conv + tanh).

Strategy (per core, data-parallel over batch: 1 sample/core on 8 cores):
  - SE gate: spatial mean -> fc1 -> relu -> fc2 -> sigmoid -> channel scale.
  - Offset conv: 9 shifted bf16 matmuls accumulating in PSUM.
  - Deform conv: bilinear sampling with |offset|<1 decomposes EXACTLY into a
    static 3x3-tap stencil per kernel position with data-dependent weights
    wy in {relu(-dy), 1-|dy|, relu(dy)} (x) wx likewise.  Each of the 81
    (k, r, s) terms is:  out += W_k @ (map_t (*) xs_shifted).
  - Perf structure:
    * tap maps stored k-major in DRAM per quarter; ONE batched
      partition-broadcast DMA per (k, quarter).
    * two bf16 copies of the padded input (even/odd column phase) keep every
      DVE modulation multiply 4B-aligned -> 2x_1P mode.
    * Z_KS kernel positions accumulate their 9 taps on DVE (z-tiles) and hit
      the PE once; their DVE chains are interleaved into the per-tap multiply
      stream so the PE never starves on them.
    * preamble (offset conv -> tap maps) for quarter q+1 is issued between
      main-loop quarters so it overlaps; PE stays warm.
  - Epilogue: tanh(psum + b_conv) fused on ACT -> DMA out.
"""

import os
import sys
from contextlib import ExitStack

import numpy as np

sys.path.insert(0, "/opt/trn_rl_repo")

import concourse.bass as bass
import concourse.bacc as bacc
import concourse.mybir as mybir
import concourse.tile as tile
from concourse.masks import make_identity

B, C, O, H, W = 8, 256, 256, 56, 56
KH = KW = 3
K2 = 9
HP, WP = H + 4, W + 4  # zero-padded by 2 for the 5x5 shift range
HW = H * W
QROWS = 14            # rows per quarter
QN = HW // 4          # 784 spatial positions per quarter
NN = QN // 2          # 392 = matmul N-chunk (fits one PSUM bank)
RED = 16              # SE reduction dim

# kernel positions handled via z-accumulation (DVE) instead of per-tap matmuls
Z_KS = (0, 8)
Z_DVE_TAPS = 9   # all z adds on DVE
TAP_KS = tuple(k for k in range(K2) if k not in Z_KS)

F32 = mybir.dt.float32
BF16 = mybir.dt.bfloat16
AF = mybir.ActivationFunctionType
ALU = mybir.AluOpType

LAST_RESULT = None


def _bcast_ap(base, extra_dims):
    """AP reading `base` ([128, N]) with extra broadcast/reshape free dims."""
    return bass.AP(tensor=base.tensor, offset=base.offset,
                   ap=[list(base.ap[0])] + [list(d) for d in extra_dims])


def build():
    nc = bacc.Bacc()
    x_d = nc.dram_tensor("x", (1, C, H, W), F32, kind="ExternalInput")
    woff_d = nc.dram_tensor("w_off", (2 * K2, C, KH, KW), F32, kind="ExternalInput")
    boff_d = nc.dram_tensor("b_off", (2 * K2,), F32, kind="ExternalInput")
    wconv_d = nc.dram_tensor("w_conv", (O, C, KH, KW), F32, kind="ExternalInput")
    bconv_d = nc.dram_tensor("b_conv", (O,), F32, kind="ExternalInput")
    fc1_d = nc.dram_tensor("fc1", (RED, C), F32, kind="ExternalInput")
    fc2_d = nc.dram_tensor("fc2", (C, RED), F32, kind="ExternalInput")
    out_d = nc.dram_tensor("out", (1, O, H, W), F32, kind="ExternalOutput")

    with tile.TileContext(nc) as tc, ExitStack() as ctx:
        singles = ctx.enter_context(tc.tile_pool(name="singles", bufs=1))
        wyxpool = ctx.enter_context(tc.tile_pool(name="wyxpool", bufs=2))
        mpool = ctx.enter_context(tc.tile_pool(name="mpool", bufs=8))
        mgpool = ctx.enter_context(tc.tile_pool(name="mgpool", bufs=6))
        reppool = ctx.enter_context(tc.tile_pool(name="reppool", bufs=2))
        zreppool = ctx.enter_context(tc.tile_pool(name="zreppool", bufs=1))
        zpool = ctx.enter_context(tc.tile_pool(name="zpool", bufs=2))
        outpool = ctx.enter_context(tc.tile_pool(name="outpool", bufs=2))
        mappool = ctx.enter_context(tc.tile_pool(name="mappool", bufs=2))
        dpool = ctx.enter_context(tc.tile_pool(name="dpool", bufs=1, space="DRAM"))
        psum_pre = ctx.enter_context(tc.tile_pool(name="psum_pre", bufs=1, space="PSUM"))
        psum_main = ctx.enter_context(tc.tile_pool(name="psum_main", bufs=1, space="PSUM"))

        # ---- static tiles ----
        xs_bf = singles.tile([128, 2, HP, WP], BF16)     # padded, scaled, even phase
        xs_bf1 = singles.tile([128, 2, HP, WP], BF16)    # odd phase: col c = col c+1
        wT = singles.tile([128, 2, K2, O], BF16)         # [c, cc, k, o]
        # woffT free dim: [0:9]=dy weights, [32:41]=dx weights (aligned blocks)
        woffT = singles.tile([128, 2, K2, 64], BF16)
        fc1T = singles.tile([128, 2, RED], F32)
        fc2T = singles.tile([128, C], F32)
        bconv = singles.tile([128, 2], F32)
        boff = singles.tile([64, 1], F32)
        y_se = singles.tile([128, 2, 1], F32)
        h_se = singles.tile([128, 1], F32)
        s_se = singles.tile([128, 2, 1], F32)
        boffn = singles.tile([64, 1], F32)
        ident = singles.tile([128, 128], BF16)
        wnat2 = singles.tile([128, 2, C * K2], BF16)
        woff_nat2 = singles.tile([2 * K2, C * K2], BF16)
        fc1Tb = singles.tile([128, 2, RED], F32)
        fc2Tb = singles.tile([128, C], F32)
        # k-major tap maps per quarter: row k*9 + (3r+s)
        maps_dram = [dpool.tile([K2 * K2, QN], BF16, name=f"maps{q}") for q in range(4)]

        make_identity(nc, ident[:, :])

        # ---- input DMA + weight DMA (cast to bf16 during DMA on SWDGE) ----
        nc.vector.memset(xs_bf[:, :, :, :], 0.0)
        x_r = x_d[:].rearrange("one c h w -> (one c) h w")
        for cc in range(2):
            nc.gpsimd.dma_start(out=xs_bf[:, cc, 2:2 + H, 2:2 + W],
                                in_=x_r[cc * 128:(cc + 1) * 128, :, :])
        wc_r = wconv_d[:].rearrange("o c kh kw -> o (c kh kw)")
        for oc in range(2):
            nc.gpsimd.dma_start(out=wnat2[:, oc, :], in_=wc_r[oc * 128:(oc + 1) * 128, :])
        nc.gpsimd.dma_start(out=woff_nat2[:, :],
                            in_=woff_d[:].rearrange("o c kh kw -> o (c kh kw)"))
        fc1_r = fc1_d[:].rearrange("m c -> c m")
        for cc in range(2):
            nc.sync.dma_start(out=fc1T[:, cc, :], in_=fc1_r[cc * 128:(cc + 1) * 128, :])
        nc.vector.memset(fc2T[:, :], 0.0)
        nc.sync.dma_start(out=fc2T[0:RED, :], in_=fc2_d[:].rearrange("c m -> m c"))
        nc.sync.dma_start(out=bconv[:, :],
                          in_=bconv_d[:].rearrange("(a c) -> c a", a=2))
        # b_off loaded de-interleaved: dy biases -> rows 0:9, dx -> rows 32:41
        nc.vector.memset(boff[:, :], 0.0)
        boff_src = boff_d[:]
        nc.sync.dma_start(out=boff[0:K2, 0:1],
                          in_=bass.AP(tensor=boff_src.tensor, offset=boff_src.offset,
                                      ap=[[2, K2], [0, 1]]))
        nc.sync.dma_start(out=boff[32:32 + K2, 0:1],
                          in_=bass.AP(tensor=boff_src.tensor,
                                      offset=boff_src.offset + 1,
                                      ap=[[2, K2], [0, 1]]))

        nc.vector.memset(woffT[:, :, :, :], 0.0)

        # ---- transpose conv weights on PE: wT[c, cc, k, o] ----
        for kk in range(K2):
            for cc in range(2):
                for oc in range(2):
                    tp = psum_pre.tile([128, 128], BF16, tag="tp")
                    src = wnat2[:, oc, :].rearrange("p (c k) -> p c k", k=K2)
                    nc.tensor.transpose(tp[:, :], src[:, cc * 128:(cc + 1) * 128, kk],
                                        ident[:, :])
                    nc.vector.tensor_copy(wT[:, cc, kk, oc * 128:(oc + 1) * 128],
                                          tp[:, :])
                tp = psum_pre.tile([128, 128], BF16, tag="tp")
                srco = woff_nat2[:, :].rearrange("p (c k) -> p c k", k=K2)
                nc.tensor.transpose(tp[:, 0:2 * K2],
                                    srco[:, cc * 128:(cc + 1) * 128, kk],
                                    ident[0:2 * K2, 0:2 * K2])
                # de-interleave offset channels: dy -> cols 0:9, dx -> cols 32:41
                nc.vector.tensor_copy(woffT[:, cc, kk, 0:K2], tp[:, 0:2 * K2:2])
                nc.vector.tensor_copy(woffT[:, cc, kk, 32:32 + K2],
                                      tp[:, 1:2 * K2:2])

        # ---- SE gate (mean over unscaled x, then scale xs in place) ----
        for cc in range(2):
            nc.vector.tensor_reduce(out=y_se[:, cc, 0:1], in_=xs_bf[:, cc, :, :],
                                    axis=mybir.AxisListType.XY, op=ALU.add)
        nc.vector.tensor_scalar_mul(y_se[:, :, 0:1], y_se[:, :, 0:1], 1.0 / HW)
        nc.vector.tensor_copy(fc1Tb[:, :, :], fc1T[:, :, :])
        nc.vector.tensor_copy(fc2Tb[:, :], fc2T[:, :])
        h_ps = psum_pre.tile([128, RED], F32, tag="se")
        for cc in range(2):
            nc.tensor.matmul(h_ps[0:RED, 0:1], lhsT=fc1Tb[:, cc, :], rhs=y_se[:, cc, 0:1],
                             start=(cc == 0), stop=(cc == 1))
        nc.vector.memset(h_se[:, :], 0.0)
        nc.vector.tensor_relu(h_se[0:RED, 0:1], h_ps[0:RED, 0:1])
        for cc in range(2):
            s_ps = psum_pre.tile([128, RED], F32, tag="se")
            nc.tensor.matmul(s_ps[:, 0:1], lhsT=fc2Tb[:, cc * 128:(cc + 1) * 128],
                             rhs=h_se[:, 0:1], start=True, stop=True)
            nc.scalar.activation(s_se[:, cc, 0:1], s_ps[:, 0:1], AF.Sigmoid)
        for cc in range(2):
            nc.vector.tensor_scalar_mul(xs_bf[:, cc, :, :], xs_bf[:, cc, :, :],
                                        s_se[:, cc, 0:1])
        # odd column phase (cols 0..57 used by tap windows; col 58/59 never read)
        nc.vector.tensor_copy(xs_bf1[:, :, :, 0:WP - 1], xs_bf[:, :, :, 1:WP])

        nc.scalar.activation(boffn[:, 0:1], boff[:, 0:1], AF.Copy, scale=-1.0)

        # ---- per-quarter preamble: offset conv -> tap-weight maps -> DRAM ----
        def preamble(q):
            wy0 = wyxpool.tile([K2, QN], BF16, tag="wy0")
            wy1 = wyxpool.tile([K2, QN], BF16, tag="wy1")
            wy2 = wyxpool.tile([K2, QN], BF16, tag="wy2")
            wx0 = wyxpool.tile([K2, QN], BF16, tag="wx0")
            wx1 = wyxpool.tile([K2, QN], BF16, tag="wx1")
            wx2 = wyxpool.tile([K2, QN], BF16, tag="wx2")
            for nn in range(2):
                off_ps = psum_pre.tile([64, NN], F32, tag="off")
                for kk in range(K2):
                    ki, kj = divmod(kk, 3)
                    dh, dw = ki - 1, kj - 1
                    for cc in range(2):
                        r0 = 2 + dh + q * QROWS + nn * (QROWS // 2)
                        rhs = xs_bf[:, cc, r0:r0 + QROWS // 2, 2 + dw:2 + dw + W]
                        nc.tensor.matmul(off_ps[0:64, :],
                                         lhsT=woffT[:, cc, kk, 0:64], rhs=rhs,
                                         start=(kk == 0 and cc == 0),
                                         stop=(kk == K2 - 1 and cc == 1))
                # offset = psum + b_off, fused into relu(+-offset) tap weights
                nsl = slice(nn * NN, (nn + 1) * NN)
                nc.scalar.activation(wy0[:, nsl], off_ps[0:K2, :], AF.Relu,
                                     scale=-1.0, bias=boffn[0:K2, 0:1])
                nc.scalar.activation(wy2[:, nsl], off_ps[0:K2, :], AF.Relu,
                                     scale=1.0, bias=boff[0:K2, 0:1])
                nc.scalar.activation(wx0[:, nsl], off_ps[32:32 + K2, :], AF.Relu,
                                     scale=-1.0, bias=boffn[32:32 + K2, 0:1])
                nc.scalar.activation(wx2[:, nsl], off_ps[32:32 + K2, :], AF.Relu,
                                     scale=1.0, bias=boff[32:32 + K2, 0:1])
            # wy1 = 1 - |dy| = 1 - (relu(dy) + relu(-dy)); same for wx1
            nc.vector.tensor_add(wy1[:, :], wy0[:, :], wy2[:, :])
            nc.scalar.activation(wy1[:, :], wy1[:, :], AF.Copy, scale=-1.0, bias=1.0)
            nc.vector.tensor_add(wx1[:, :], wx0[:, :], wx2[:, :])
            nc.scalar.activation(wx1[:, :], wx1[:, :], AF.Copy, scale=-1.0, bias=1.0)
            wys = (wy0, wy1, wy2)
            wxs = (wx0, wx1, wx2)
            md = maps_dram[q][0:1, 0:1]

            def make_prod(r, s):
                def op():
                    rs = 3 * r + s
                    mtmp = mappool.tile([K2, QN], BF16, name="mtmp")
                    nc.vector.tensor_mul(mtmp[:, :], wys[r][:, :], wxs[s][:, :])
                    nc.sync.dma_start(
                        out=bass.AP(tensor=md.tensor, offset=md.offset + rs * QN,
                                    ap=[[K2 * QN, K2], [1, QN]]),
                        in_=mtmp[:, :])
                return op
            return [make_prod(r, s) for r in range(3) for s in range(3)]

        # ---- main deform-conv loop for one quarter ----
        n_events = len(TAP_KS) * K2 + len(Z_KS)

        def mod_mul(out_ap, mrep_row, kk, r, s, q, engine=None):
            ki, kj = divmod(kk, 3)
            dh, dw = ki - 1 + r - 1, kj - 1 + s - 1
            r0 = 2 + dh + q * QROWS
            cs = 2 + dw
            if cs % 2 == 0:
                xs_win = xs_bf[:, :, r0:r0 + QROWS, cs:cs + W]
            else:
                xs_win = xs_bf1[:, :, r0:r0 + QROWS, cs - 1:cs - 1 + W]
            mrep_b = _bcast_ap(mrep_row, [[0, 2], [W, QROWS], [1, W]])
            (engine or nc.vector).tensor_tensor(out_ap, xs_win, mrep_b, op=ALU.mult)

        def main_quarter(q, extra_ops=()):
            ps = [psum_main.tile([128, QN], F32, tag=f"ps{oc}", name=f"ps{oc}")
                  for oc in range(2)]

            def do_mms(rhs_tile, kk, ev):
                for cc in range(2):
                    for oc in range(2):
                        for n0, n1 in ((0, 512), (512, QN)):
                            nc.tensor.matmul(
                                ps[oc][:, n0:n1],
                                lhsT=wT[:, cc, kk, oc * 128:(oc + 1) * 128],
                                rhs=rhs_tile[:, cc, n0:n1],
                                start=(ev == 0 and cc == 0),
                                stop=(ev == n_events - 1 and cc == 1))

            md = maps_dram[q][0:1, 0:1]

            def bcast(out_ap, kk):
                nc.gpsimd.dma_start(
                    out=out_ap,
                    in_=bass.AP(tensor=md.tensor, offset=md.offset + kk * K2 * QN,
                                ap=[[0, 128], [QN, K2], [1, QN]]))

            # z-map broadcasts + per-tap k lookahead broadcast queue
            zrep = zreppool.tile([128, len(Z_KS), K2, QN], BF16, name="zrep")
            for zi, zk in enumerate(Z_KS):
                bcast(zrep[:, zi, :, :], zk)
            reps = {}
            reps[TAP_KS[0]] = reppool.tile([128, K2, QN], BF16, name="rep")
            bcast(reps[TAP_KS[0]][:, :, :], TAP_KS[0])

            # build deferred z-op list: [(is_first_mult?, zi, zk, r, s)]
            zaccs = {}
            for zi, zk in enumerate(Z_KS):
                zaccs[zi] = zpool.tile([128, 2, QN], BF16, name=f"zacc{zi}")
            zops = []
            for zi, zk in enumerate(Z_KS):
                for t in range(K2):
                    zops.append((zi, zk, t))

            def issue_zop(idx):
                zi, zk, t = zops[idx]
                r, s = divmod(t, 3)
                acc = zaccs[zi]
                if t == 0:
                    acc_v = acc[:, :, :].rearrange("p a (r c) -> p a r c", c=W)
                    mod_mul(acc_v, zrep[:, zi, t, :], zk, r, s, q)
                elif t < Z_DVE_TAPS:
                    m = mpool.tile([128, 2, QN], BF16)
                    m_v = m[:, :, :].rearrange("p a (r c) -> p a r c", c=W)
                    mod_mul(m_v, zrep[:, zi, t, :], zk, r, s, q)
                    nc.vector.tensor_add(acc[:, :, :], acc[:, :, :], m[:, :, :])
                else:
                    m = mgpool.tile([128, 2, QN], BF16, name="mg")
                    m_v = m[:, :, :].rearrange("p a (r c) -> p a r c", c=W)
                    mod_mul(m_v, zrep[:, zi, t, :], zk, r, s, q)
                    nc.vector.tensor_add(acc[:, :, :], acc[:, :, :], m[:, :, :])

            # interleave: hold z-ops for the first Z_DELAY taps (zrep DMA still
            # in flight), then catch up evenly so chains finish ~5 taps early
            Z_DELAY = 10
            n_taps = len(TAP_KS) * K2
            zdone = 0
            ev = 0
            tap_i = 0
            for ki, kk in enumerate(TAP_KS):
                if ki + 1 < len(TAP_KS):
                    nk = TAP_KS[ki + 1]
                    reps[nk] = reppool.tile([128, K2, QN], BF16, name="rep")
                    bcast(reps[nk][:, :, :], nk)
                for t in range(K2):
                    r, s = divmod(t, 3)
                    m = mpool.tile([128, 2, QN], BF16)
                    m_v = m[:, :, :].rearrange("p a (r c) -> p a r c", c=W)
                    mod_mul(m_v, reps[kk][:, t, :], kk, r, s, q)
                    do_mms(m, kk, ev)
                    ev += 1
                    tap_i += 1
                    if tap_i > Z_DELAY:
                        ztarget = min(len(zops),
                                      (len(zops) * (tap_i - Z_DELAY)) // (n_taps - Z_DELAY - 5) if n_taps - Z_DELAY - 5 > 0 else len(zops))
                        while zdone < ztarget:
                            issue_zop(zdone)
                            zdone += 1
            while zdone < len(zops):
                issue_zop(zdone)
                zdone += 1
            for zi, zk in enumerate(Z_KS):
                do_mms(zaccs[zi], zk, ev)
                ev += 1
            assert ev == n_events

            out_r = out_d[:].rearrange("one o h w -> (one o) h w")
            for oc in range(2):
                osb = outpool.tile([128, QN], F32)
                nc.scalar.activation(osb[:, :], ps[oc][:, :], AF.Tanh,
                                     bias=bconv[:, oc:oc + 1])
                nc.sync.dma_start(
                    out=out_r[oc * 128:(oc + 1) * 128,
                              q * QROWS:(q + 1) * QROWS, :],
                    in_=osb[:, :])

        # software pipeline: preamble(q+1) issued between main quarters
        prods = preamble(0)
        for op in prods:
            op()
        prods = preamble(1)
        for op in prods:
            op()
        main_quarter(0)
        prods = preamble(2)
        for op in prods:
            op()
        main_quarter(1)
        prods = preamble(3)
        for op in prods:
            op()
        main_quarter(2)
        main_quarter(3)
    nc.finalize()
    return nc


_NC = None


def _get_nc():
    global _NC
    if _NC is None:
        _NC = build()
    return _NC


def kernel(**inputs):
    global LAST_RESULT
    from concourse.bass_utils import run_bass_kernel_spmd

    nc = _get_nc()
    x = np.ascontiguousarray(inputs["x"], dtype=np.float32)
    shared = {k: np.ascontiguousarray(np.asarray(inputs[k]), dtype=np.float32)
              for k in ("w_off", "b_off", "w_conv", "b_conv", "fc1", "fc2")}
    in_maps = [{"x": x[i:i + 1], **shared} for i in range(B)]
    res = run_bass_kernel_spmd(nc, in_maps, core_ids=list(range(B)),
                               trace=bool(int(os.environ.get("KB_TRACE", "0"))))
    LAST_RESULT = res
    out = np.concatenate([res.results[i]["out"] for i in range(B)], axis=0)
    return out.astype(np.float32)


if __name__ == "__main__":
    nc = build()
    print("build OK")
